# revision 1
# baseline (speedup 1.0000x reference)
"""Trainium2 Bass kernel for nn_CrossFeature (sparse_attention).

Math (per batch b):
    att[b,n,f]  = (x[b] @ W.T @ q.T).T * E**-0.5          # folded: x[b] @ (qW).T
    Xs          = 0.5 * att                               # entmax15 pre-scale
    gate        = entmax15(att) over f  (solved by Newton on the entmax root)
    out[b,n,e]  = exp( sum_f gate*value * x[b,f,e] )

Key algebraic moves:
  * stage-1/2 fused: qtilde = (q @ W) * 0.5 * E**-0.5, Xs = x @ qtilde.T
  * entmax15 bisection (50 iters) replaced by Newton on
        g(tau) = sum_f relu(Xs-tau)^2 - 1,
    with moments from bn_stats over m = max(Xs, tau):
        s1 = sum relu(Xs-tau)   = 32*((mean_e-tau)+(mean_o-tau))
        s2 = sum relu(Xs-tau)^2 = M2_e + M2_o + 32*((mean_e-tau)^2+(mean_o-tau)^2)
    init tau0 = mean - (cbar/2 + (1 - v64)/(128*cbar))  (linearized sqrt)
    3 Newton iterations reach fp32 roundoff (validated vs the reference).

Sharding: pure data-parallel, batch 2048 -> 8 cores x 256.
"""

import numpy as np

B_FULL, F, E, N = 2048, 64, 256, 64
NCORES = 8
B_LOC = B_FULL // NCORES

SCALE = 0.5 * (E ** -0.5)   # folds entmax's (alpha-1) into qtilde
CBAR = 0.097                # linearization point for sqrt((1-v64)/64)
NEWTON_ITERS = 3


def build_program(B_loc=B_LOC, NG=4):
    import concourse.tile as tile
    from concourse import bacc, mybir, masks

    f32 = mybir.dt.float32
    bf16 = mybir.dt.bfloat16
    Alu = mybir.AluOpType
    ACTF = mybir.ActivationFunctionType

    HALF = B_loc // 2
    C = HALF // NG            # batch-pairs per group
    assert C * NG == HALF and C % 4 == 0
    FSEG = 512 // F           # bn_stats segments per instruction (8)

    nc = bacc.Bacc("TRN2", debug=False, num_devices=NCORES)
    x_d = nc.dram_tensor("x", [B_loc, F, E], f32, kind="ExternalInput").ap()
    w_d = nc.dram_tensor("bilinear_w", [E, E], f32, kind="ExternalInput").ap()
    q_d = nc.dram_tensor("query", [N, E], f32, kind="ExternalInput").ap()
    v_d = nc.dram_tensor("value", [N, F], f32, kind="ExternalInput").ap()
    o_d = nc.dram_tensor("out", [B_loc, N, E], f32, kind="ExternalOutput").ap()

    K0 = 0.5 * CBAR + 1.0 / (128.0 * CBAR)
    KW = 1.0 / (128.0 * CBAR)

    with tile.TileContext(nc) as tc:
        with (
            tc.tile_pool(name="const", bufs=1) as constp,
            tc.tile_pool(name="xp", bufs=C + C // 2 + 2) as xpp,
            tc.tile_pool(name="xbf", bufs=6) as xbfp,
            tc.tile_pool(name="xt", bufs=10) as xtp,
            tc.tile_pool(name="xs", bufs=2) as xsp,
            tc.tile_pool(name="mb", bufs=2) as mbp,
            tc.tile_pool(name="aw", bufs=2) as awp,
            tc.tile_pool(name="st", bufs=2) as stp,
            tc.tile_pool(name="sm", bufs=3) as smp,
            tc.tile_pool(name="awt", bufs=3) as awtp,
            tc.tile_pool(name="osb", bufs=3) as osbp,
            tc.tile_pool(name="ps12", bufs=2, space="PSUM") as ps12p,
            tc.tile_pool(name="pst", bufs=2, space="PSUM") as pstp,
            tc.tile_pool(name="ps3", bufs=3, space="PSUM") as ps3p,
        ):
            # ---------------- constants ----------------
            ident = constp.tile([128, 128], f32)
            masks.make_identity(nc, ident[:])

            v2 = constp.tile([128, F], f32)
            nc.sync.dma_start(v2[0:64, :], v_d[:, :])
            nc.sync.dma_start(v2[64:128, :], v_d[:, :])

            wt = {}
            for di in range(2):
                for ej in range(2):
                    t = constp.tile([128, 128], f32, tag=f"wt{di}{ej}")
                    nc.sync.dma_start(
                        t[:], w_d[di * 128:(di + 1) * 128, ej * 128:(ej + 1) * 128]
                    )
                    wt[di, ej] = t

            qtin = []
            for di in range(2):
                t = constp.tile([128, N], f32, tag=f"qtin{di}")
                nc.sync.dma_start(
                    t[:], q_d[:, di * 128:(di + 1) * 128].transpose([1, 0])
                )
                qtin.append(t)

            # qtilde^T = W.T-contract: qt[e, n] = sum_d W[d, e] q[n, d], then * SCALE
            qt_bf = []
            for ej in range(2):
                ps = ps12p.tile([128, N], f32, tag="ps12")
                for di in range(2):
                    nc.tensor.matmul(
                        ps[:], wt[di, ej][:], qtin[di][:],
                        start=(di == 0), stop=(di == 1),
                    )
                t = constp.tile([128, N], bf16, tag=f"qtbf{ej}")
                nc.scalar.mul(t[:], ps[:], SCALE)
                qt_bf.append(t)

            # ---------------- per-group pipeline ----------------
            for g in range(NG):
                b0 = g * C                      # first half-1 batch of group
                # --- load x pairs, cast to bf16, transpose via xbar DMA ---
                xp_tiles = []
                xt_tiles = []
                for c in range(C):
                    bA = b0 + c
                    xp = xpp.tile([128, E], f32, tag="xp")
                    nc.sync.dma_start(xp[0:64, :], x_d[bA, :, :])
                    nc.sync.dma_start(xp[64:128, :], x_d[bA + HALF, :, :])
                    xp_tiles.append(xp)
                    xbf = xbfp.tile([128, E], bf16, tag="xbf")
                    nc.gpsimd.tensor_copy(xbf[:], xp[:])
                    pair_t = []
                    for ec in range(2):
                        xt = xtp.tile([128, 128], bf16, tag="xt")
                        nc.sync.dma_start_transpose(
                            xt[:], xbf[:, ec * 128:(ec + 1) * 128]
                        )
                        pair_t.append(xt)
                    xt_tiles.append(pair_t)

                # --- stage-12 matmuls: Xs[p=(n|n), c*64+f] ---
                xs_t = xsp.tile([128, C, 72], f32, tag="xs")
                xs3 = xs_t[:, :, 0:F]
                for blk in range(C // 8):
                    ps = ps12p.tile([128, 512], f32, tag="ps12")
                    for s in range(8):
                        c = blk * 8 + s
                        for ec in range(2):
                            nc.tensor.matmul(
                                ps[0:64, s * 64:(s + 1) * 64],
                                qt_bf[ec][:],
                                xt_tiles[c][ec][:, 0:64],
                                start=(ec == 0), stop=(ec == 1),
                                tile_position=(0, 0),
                                skip_group_check=True,
                            )
                        for ec in range(2):
                            nc.tensor.matmul(
                                ps[64:128, s * 64:(s + 1) * 64],
                                qt_bf[ec][:],
                                xt_tiles[c][ec][:, 64:128],
                                start=(ec == 0), stop=(ec == 1),
                                tile_position=(0, 64),
                                skip_group_check=True,
                            )
                    nc.scalar.copy(
                        xs3[:, blk * 8:(blk + 1) * 8, :],
                        ps[:].rearrange("p (c f) -> p c f", f=F),
                    )

                # --- entmax via Newton ---
                st = stp.tile([128, C, 8], f32, tag="st")

                def bn_pass(src3):
                    # HW BNStats: one segment per instruction (out = 6/partition)
                    for c in range(C):
                        nc.vector.bn_stats(
                            st[:, c, 0:6],
                            src3[:, c, :],
                        )

                def sl(k):
                    return st[:, :, k:k + 1]        # [128, C, 1]

                tau = smp.tile([128, C], f32, tag="tau")
                tauu = tau[:].unsqueeze(2)          # [128, C, 1]

                # init: tau0 = 0.5*msum + KW*wsum - K0
                bn_pass(xs3)
                msum = smp.tile([128, C], f32, tag="msum")
                wsum = smp.tile([128, C], f32, tag="wsum")
                nc.vector.tensor_add(msum[:].unsqueeze(2), sl(1), sl(4))
                nc.vector.tensor_add(wsum[:].unsqueeze(2), sl(2), sl(5))
                nc.vector.tensor_scalar(
                    out=msum[:], in0=msum[:], scalar1=0.5, scalar2=K0,
                    op0=Alu.mult, op1=Alu.subtract,
                )
                nc.vector.scalar_tensor_tensor(
                    out=tau[:], in0=wsum[:], scalar=KW, in1=msum[:],
                    op0=Alu.mult, op1=Alu.add,
                )

                mb_t = mbp.tile([128, C, 72], f32, tag="mb")
                mb3 = mb_t[:, :, 0:F]
                taub = tauu.broadcast_to([128, C, F])

                a2 = smp.tile([128, C, 2], f32, tag="a2")
                u2 = smp.tile([128, C, 2], f32, tag="u2")
                s1m = smp.tile([128, C], f32, tag="s1m")
                s2s = smp.tile([128, C], f32, tag="s2s")
                rcp = smp.tile([128, C], f32, tag="rcp")

                for it in range(NEWTON_ITERS + 1):
                    nc.vector.tensor_max(mb3, xs3, taub)
                    bn_pass(mb3)
                    # a = mean - tau (even, odd)
                    nc.vector.tensor_sub(
                        a2[:, :, 0:1], sl(1), tauu
                    )
                    nc.vector.tensor_sub(
                        a2[:, :, 1:2], sl(4), tauu
                    )
                    # sq = a*a on ACT; u = 32*sq + M2
                    nc.scalar.square(u2[:], a2[:])
                    nc.vector.scalar_tensor_tensor(
                        out=u2[:, :, 0:1], in0=u2[:, :, 0:1], scalar=32.0,
                        in1=sl(2), op0=Alu.mult, op1=Alu.add,
                    )
                    nc.vector.scalar_tensor_tensor(
                        out=u2[:, :, 1:2], in0=u2[:, :, 1:2], scalar=32.0,
                        in1=sl(5), op0=Alu.mult, op1=Alu.add,
                    )
                    nc.vector.tensor_reduce(
                        s2s[:], u2[:], axis=mybir.AxisListType.X, op=Alu.add,
                    )
                    if it < NEWTON_ITERS:
                        # dtau = (s2 - 1) / (64 * s1m);  s1m = ae + ao
                        nc.vector.tensor_reduce(
                            s1m[:], a2[:], axis=mybir.AxisListType.X, op=Alu.add,
                        )
                        nc.vector.reciprocal(rcp[:], s1m[:])
                        nc.vector.tensor_scalar(
                            out=s2s[:], in0=s2s[:], scalar1=-1.0, scalar2=None,
                            op0=Alu.add,
                        )
                        nc.vector.tensor_mul(s2s[:], s2s[:], rcp[:])
                        nc.vector.scalar_tensor_tensor(
                            out=tau[:], in0=s2s[:], scalar=1.0 / 64.0, in1=tau[:],
                            op0=Alu.mult, op1=Alu.add,
                        )

                # final: recip_s2, d = m - tau, aw = d^2 * v
                recs2 = smp.tile([128, C], f32, tag="recs2")
                nc.vector.reciprocal(recs2[:], s2s[:])
                nc.vector.tensor_sub(mb3, mb3, taub)
                aw_t = awp.tile([128, C * F], f32, tag="aw")
                aw3 = aw_t[:].rearrange("p (c f) -> p c f", f=F)
                nc.scalar.square(aw3, mb3)
                v2b = v2[:].unsqueeze(1).broadcast_to([128, C, F])
                nc.vector.tensor_mul(aw3, aw3, v2b)

                # --- stage-3: out[b] = exp( (aw_b)^T-weights @ x_b * 1/s2 ) ---
                for blk in range(C // 4):
                    # One full transpose per pair: S_c [128,64] -> [64,128] at
                    # PSUM partition 0 (HW requires transpose out/in base 0).
                    # Slot s holds [W1 | W2] at cols s*128 : s*128+128.
                    pst = pstp.tile([64, 512], f32, tag="pst")
                    for s in range(4):
                        c = blk * 4 + s
                        nc.tensor.transpose(
                            pst[0:64, s * 128:(s + 1) * 128],
                            aw_t[:, c * 64:(c + 1) * 64],
                            ident[:],
                        )
                    pst3 = pst[:].rearrange("p (s h f) -> p s h f", s=4, h=2)
                    awt = awtp.tile([128, 256], f32, tag="awt")
                    awt3 = awt[:].rearrange("p (s f) -> p s f", s=4)
                    nc.scalar.copy(awt3[0:64, :, :], pst3[:, :, 0, :])
                    # W2 must reach partitions 64:128: evac to SBUF first,
                    # then partition-shift with an SBUF->SBUF DMA.
                    awt_tmp = awtp.tile([64, 256], f32, tag="awt_tmp")
                    nc.scalar.copy(
                        awt_tmp[:].rearrange("p (s f) -> p s f", s=4),
                        pst3[:, :, 1, :],
                    )
                    nc.sync.dma_start(awt[64:128, :], awt_tmp[:])

                    osb = osbp.tile([128, 1024], f32, tag="osb")
                    for s in range(4):
                        c = blk * 4 + s
                        ps3 = ps3p.tile([128, E], f32, tag="ps3")
                        nc.tensor.matmul(
                            ps3[0:64, :],
                            awt[0:64, s * 64:(s + 1) * 64],
                            xp_tiles[c][0:64, :],
                            start=True, stop=True,
                            tile_position=(0, 0),
                            skip_group_check=True,
                        )
                        nc.tensor.matmul(
                            ps3[64:128, :],
                            awt[64:128, s * 64:(s + 1) * 64],
                            xp_tiles[c][64:128, :],
                            start=True, stop=True,
                            tile_position=(64, 64),
                            skip_group_check=True,
                        )
                        nc.scalar.activation(
                            osb[:, s * 256:(s + 1) * 256], ps3[:],
                            ACTF.Exp, scale=recs2[:, c:c + 1],
                        )
                    bA = b0 + blk * 4
                    nc.sync.dma_start(
                        o_d[bA:bA + 4, :, :].transpose([1, 0, 2]),
                        osb[0:64, :].rearrange("p (t e) -> p t e", t=4),
                    )
                    nc.sync.dma_start(
                        o_d[HALF + bA:HALF + bA + 4, :, :].transpose([1, 0, 2]),
                        osb[64:128, :].rearrange("p (t e) -> p t e", t=4),
                    )
    if not nc.is_finalized():
        nc.finalize()
    return nc


_NC_CACHE = {}


def _get_program(B_loc, NG):
    key = (B_loc, NG)
    if key not in _NC_CACHE:
        _NC_CACHE[key] = build_program(B_loc, NG)
    return _NC_CACHE[key]


def kernel(**inputs):
    from concourse.bass_utils import run_bass_kernel_spmd

    x = np.ascontiguousarray(np.asarray(inputs["x"], dtype=np.float32))
    w = np.ascontiguousarray(np.asarray(inputs["bilinear_w"], dtype=np.float32))
    q = np.ascontiguousarray(np.asarray(inputs["query"], dtype=np.float32))
    v = np.ascontiguousarray(np.asarray(inputs["value"], dtype=np.float32))
    B = x.shape[0]
    B_loc = B // NCORES

    nc = _get_program(B_loc, 4)

    in_maps = []
    for core in range(NCORES):
        sh = x[core * B_loc:(core + 1) * B_loc]
        in_maps.append(
            {"x": np.ascontiguousarray(sh), "bilinear_w": w, "query": q, "value": v}
        )

    import os
    trace = bool(int(os.environ.get("KERNEL_TRACE", "0")))
    res = run_bass_kernel_spmd(
        nc, in_maps, core_ids=list(range(NCORES)), trace=trace,
        trace_cores=[0] if trace else None,
    )
    if trace:
        kernel.last_exec_time_ns = res.exec_time_ns
        kernel.last_trace = res.instructions_and_trace
    out = np.concatenate([r["out"] for r in res.results], axis=0)
    return out


if __name__ == "__main__":
    # smoke-test the builder only
    nc = build_program(32, 2)
    print("build ok:", len(nc.inst_map), "instructions")



# revision 11
# speedup vs baseline: 2.6250x; 2.6250x over previous
"""Trainium2 Bass kernel for nn_CrossFeature (sparse_attention).

Math (per batch b):
    att[b,n,f]  = (x[b] @ W.T @ q.T).T * E**-0.5
    Xs          = 0.5 * att                               # entmax15 pre-scale
    gate        = entmax15(att) over f  (Newton on the entmax root)
    out[b,n,e]  = exp( sum_f gate*value * x[b,f,e] )

v3 design: all-fp32 storage, float32r (TF32-ish) matmuls (no bf16
casts); x transposed on the PE instead of DMA-transpose; stage-1/2 as
512-col moving matmuls; entmax Newton on whole-group [128, 32*64] DVE
passes with segmented tensor_reduce instead of per-pair bn_stats.

fp32r matmuls require dst partition base 0 (no tile_position), so:
  * stage-1/2 routes the two partition halves of the Xs PSUM bank via
    zero-padded stationaries [qt|0] / [0|qt] and full-height matmuls;
  * each bank pairs batch T_c=base+c (partitions 0:64) with
    B_c=base+8+c (64:128); the gate lives in a zero-interleaved tile
    aw2[0:64, slot, 0:64]=gate(T) / [64:128, slot, 64:128]=gate(B);
    its PE transpose is block-diagonal [gT(T),0;0,gT(B)] so one
    full-height fp32r matmul computes stage-3 for both batches.
  * 1/s2 is folded into the gate so stage-3 exp needs no per-batch
    scale and runs on [128,512].

Sharding: pure data-parallel, batch 2048 -> 8 cores x 256.
"""

import numpy as np

B_FULL, F, E, N = 2048, 64, 256, 64
NCORES = 8
B_LOC = B_FULL // NCORES

SCALE = 0.5 * (E ** -0.5)   # folds entmax's (alpha-1) into qtilde
CBAR = 0.097                # linearization point for sqrt((1-v64)/64)
NEWTON_ITERS = 2


def build_program(B_loc=B_LOC, newton_iters=NEWTON_ITERS):
    import concourse.tile as tile
    from concourse import bacc, mybir, masks

    f32 = mybir.dt.float32
    f32r = mybir.dt.float32r
    Alu = mybir.AluOpType
    ACTF = mybir.ActivationFunctionType
    AX = mybir.AxisListType

    NBANKS = B_loc // 16          # 16 batches per bank (8 T + 8 B)
    NG = 4                        # groups (entmax granularity)
    BPG = NBANKS // NG            # banks per group
    G = BPG * 8                   # batch-slots per group tile (32)
    assert NBANKS % NG == 0

    K0 = 0.5 * CBAR + 1.0 / (128.0 * CBAR)
    KW = 1.0 / (128.0 * CBAR)

    nc = bacc.Bacc("TRN2", debug=False, num_devices=NCORES)
    x_d = nc.dram_tensor("x", [B_loc, F, E], f32r, kind="ExternalInput").ap()
    w_d = nc.dram_tensor("bilinear_w", [E, E], f32, kind="ExternalInput").ap()
    q_d = nc.dram_tensor("query", [N, E], f32, kind="ExternalInput").ap()
    v_d = nc.dram_tensor("value", [N, F], f32, kind="ExternalInput").ap()
    o_d = nc.dram_tensor("out", [B_loc, N, E], f32, kind="ExternalOutput").ap()

    # batch (bb, r, jj) = bb*16 + r*8 + jj; partition pair = (T_jj | B_jj)


    with tile.TileContext(nc) as tc:
        with (
            tc.tile_pool(name="const", bufs=1) as constp,
            tc.tile_pool(name="xp", bufs=4) as xpp,       # [128,8,256]
            tc.tile_pool(name="xt", bufs=4) as xtp,       # [128,1024] x(2/bank)
            tc.tile_pool(name="xsg", bufs=2) as xsgp,     # [128,G,64]
            tc.tile_pool(name="scr", bufs=2) as scrp,     # r chain
            tc.tile_pool(name="aw2", bufs=2) as aw2p,     # zero-interleaved gate
            tc.tile_pool(name="sm", bufs=2) as smp,       # [128,G] smalls
            tc.tile_pool(name="awt", bufs=3) as awtp,     # [128,1024]
            tc.tile_pool(name="osb", bufs=6) as osbp,     # [128,512]
            tc.tile_pool(name="pst", bufs=2, space="PSUM") as pstp,
            tc.tile_pool(name="ps12", bufs=2, space="PSUM") as ps12p,
            tc.tile_pool(name="psdt", bufs=2, space="PSUM") as psdtp,
            tc.tile_pool(name="ps3", bufs=2, space="PSUM") as ps3p,
        ):
            # ---------------- constants ----------------
            ident = constp.tile([128, 128], f32)
            masks.make_identity(nc, ident[:])

            v2 = constp.tile([128, F], f32)
            nc.sync.dma_start(v2[0:64, :], v_d[:, :])
            nc.sync.dma_start(v2[64:128, :], v_d[:, :])

            wt = {}
            for di in range(2):
                for ej in range(2):
                    t = constp.tile([128, 128], f32, tag=f"wt{di}{ej}")
                    nc.sync.dma_start(
                        t[:], w_d[di * 128:(di + 1) * 128, ej * 128:(ej + 1) * 128]
                    )
                    wt[di, ej] = t

            qtin = []
            for di in range(2):
                t = constp.tile([128, N], f32, tag=f"qtin{di}")
                nc.sync.dma_start(
                    t[:], q_d[:, di * 128:(di + 1) * 128].transpose([1, 0])
                )
                qtin.append(t)

            # qtilde^T[e, n] = sum_d W[d, e] q[n, d], scaled.
            # qtz[ec][h]: [128,128] stationary, h=0 -> [qt|0], h=1 -> [0|qt]
            qtz = []
            for ej in range(2):
                ps = ps12p.tile([128, 512], f32, tag="ps12")
                for di in range(2):
                    nc.tensor.matmul(
                        ps[:, 0:N], wt[di, ej][:], qtin[di][:],
                        start=(di == 0), stop=(di == 1),
                    )
                pair = []
                for h in range(2):
                    t = constp.tile([128, 128], f32r, tag=f"qtz{ej}{h}")
                    nc.scalar.mul(t[:, h * 64:h * 64 + 64], ps[:, 0:N], SCALE)
                    nc.scalar.mul(t[:, (1 - h) * 64:(1 - h) * 64 + 64],
                                  ps[:, 0:N], 0.0)
                    pair.append(t)
                qtz.append(pair)

            # aw2 buffers: zero-interleaved gate; off-quadrants stay 0 forever
            aw2_bufs = []
            for i in range(2):
                t = aw2p.tile([128, G, 128], f32, tag="aw2")
                nc.vector.memset(t[0:64, :, 64:128], 0.0)
                nc.vector.memset(t[64:128, :, 0:64], 0.0)
                aw2_bufs.append(t)

            # ---------------- per-bank phase A ----------------
            def phase_a(bank, xs_g):
                bslot = (bank % BPG) * 8
                base = bank * 16
                xp = xpp.tile([128, 8, 256], f32r, tag="xp")
                nc.sync.dma_start(
                    xp[0:64, :, :],
                    x_d[base:base + 8].rearrange("jj f e -> f jj e"),
                )
                nc.sync.dma_start(
                    xp[64:128, :, :],
                    x_d[base + 8:base + 16].rearrange("jj f e -> f jj e"),
                )

                xt = []
                for ec in range(2):
                    t = xtp.tile([128, 1024], f32r, tag="xt")
                    t4 = t[:].rearrange("p (tb jh c) -> p tb jh c", tb=2, jh=2)
                    for jh in range(2):
                        pst = pstp.tile([128, 512], f32, tag="pst")
                        pst4 = pst[:].rearrange(
                            "p (tb c f) -> p tb c f", tb=2, c=4
                        )
                        for j4 in range(4):
                            jj = jh * 4 + j4
                            nc.tensor.transpose(
                                pst4[:, :, j4, :],
                                xp[:, jj, ec * 128:(ec + 1) * 128].bitcast(f32),
                                ident[:],
                            )
                        nc.scalar.copy(t4[:, :, jh, :], pst[:])
                    xt.append(t)

                ps = ps12p.tile([128, 512], f32, tag="ps12")
                k = 0
                for h in range(2):
                    for ec in range(2):
                        nc.tensor.matmul(
                            ps[:, :],
                            qtz[ec][h][:],
                            xt[ec][:, h * 512:(h + 1) * 512],
                            start=(k == 0), stop=(k == 3),
                        )
                        k += 1
                nc.vector.tensor_copy(
                    xs_g[:, bslot:bslot + 8, :],
                    ps[:].rearrange("p (s f) -> p s f", f=F),
                )
                return xp

            # ---------------- entmax (per group) ----------------
            def entmax(xs_g, aw2):
                xs3 = xs_g[:]
                r_t = scrp.tile([128, G, F], f32, tag="r")
                r = r_t[:]
                sr = smp.tile([128, G], f32, tag="sr")
                srq = smp.tile([128, G], f32, tag="srq")
                tau = smp.tile([128, G], f32, tag="tau")
                s1 = smp.tile([128, G], f32, tag="s1")
                u = smp.tile([128, G], f32, tag="u")
                w = smp.tile([128, G], f32, tag="w")
                s2 = smp.tile([128, G], f32, tag="s2")
                rec = smp.tile([128, G], f32, tag="rec")
                dlt = smp.tile([128, G], f32, tag="dlt")
                taub = tau[:].unsqueeze(2).broadcast_to([128, G, F])

                def seg_sum(dst, src3):
                    nc.vector.tensor_reduce(
                        dst[:], src3, axis=AX.X, op=Alu.add,
                    )

                # init: tau0 = sr/64 + KW*(srq - sr^2/64) - K0
                nc.vector.tensor_mul(r, xs3, xs3)
                seg_sum(srq, r)
                seg_sum(sr, xs3)
                nc.vector.tensor_mul(u[:], sr[:], sr[:])
                nc.vector.scalar_tensor_tensor(
                    out=w[:], in0=u[:], scalar=-1.0 / 64.0, in1=srq[:],
                    op0=Alu.mult, op1=Alu.add,
                )  # w = V = srq - sr^2/64
                nc.vector.tensor_scalar(
                    out=w[:], in0=w[:], scalar1=KW, scalar2=-K0,
                    op0=Alu.mult, op1=Alu.add,
                )  # w = KW*V - K0
                nc.vector.scalar_tensor_tensor(
                    out=tau[:], in0=sr[:], scalar=1.0 / 64.0, in1=w[:],
                    op0=Alu.mult, op1=Alu.add,
                )

                for _ in range(newton_iters):
                    nc.vector.tensor_max(r, xs3, taub)
                    seg_sum(sr, r)
                    nc.vector.tensor_mul(r, r, r)
                    seg_sum(srq, r)
                    # s1 = sr - 64*tau ; s2 = srq - 2*tau*sr + 64*tau^2
                    nc.vector.scalar_tensor_tensor(
                        out=s1[:], in0=tau[:], scalar=-64.0, in1=sr[:],
                        op0=Alu.mult, op1=Alu.add,
                    )
                    nc.vector.tensor_mul(u[:], tau[:], sr[:])
                    nc.vector.tensor_mul(w[:], tau[:], tau[:])
                    nc.vector.scalar_tensor_tensor(
                        out=s2[:], in0=w[:], scalar=64.0, in1=srq[:],
                        op0=Alu.mult, op1=Alu.add,
                    )
                    nc.vector.scalar_tensor_tensor(
                        out=s2[:], in0=u[:], scalar=-2.0, in1=s2[:],
                        op0=Alu.mult, op1=Alu.add,
                    )
                    # tau += (s2 - 1) / (2*s1)
                    nc.vector.reciprocal(rec[:], s1[:])
                    nc.vector.tensor_scalar(
                        out=s2[:], in0=s2[:], scalar1=-1.0, scalar2=None,
                        op0=Alu.add,
                    )
                    nc.vector.tensor_mul(dlt[:], s2[:], rec[:])
                    nc.vector.scalar_tensor_tensor(
                        out=tau[:], in0=dlt[:], scalar=0.5, in1=tau[:],
                        op0=Alu.mult, op1=Alu.add,
                    )

                # final: d = relu(Xs - tau); aw = d^2 * v / s2, written into
                # the data quadrants of the zero-interleaved aw2
                nc.vector.tensor_max(r, xs3, taub)
                nc.vector.tensor_sub(r, r, taub)        # d
                nc.vector.tensor_mul(r, r, r)           # d^2
                seg_sum(s2, r)
                nc.vector.reciprocal(rec[:], s2[:])
                v2b = v2[:].unsqueeze(1).broadcast_to([128, G, F])
                nc.vector.tensor_mul(r, r, v2b)         # d^2 * v
                recb = rec[:].unsqueeze(2).broadcast_to([128, G, F])
                aw3 = aw2[:]
                nc.vector.tensor_mul(
                    aw3[0:64, :, 0:64], r[0:64, :, :], recb[0:64, :, :]
                )
                nc.vector.tensor_mul(
                    aw3[64:128, :, 64:128], r[64:128, :, :], recb[64:128, :, :]
                )

            # ---------------- per-bank phase C ----------------
            def phase_c(bank, aw2, xp):
                base = bank * 16
                bslot = (bank % BPG) * 8
                awt = awtp.tile([128, 1024], f32r, tag="awt")
                for jh in range(2):
                    psdt = psdtp.tile([128, 512], f32, tag="psdt")
                    for j4 in range(4):
                        nc.tensor.transpose(
                            psdt[:, j4 * 128:(j4 + 1) * 128],
                            aw2[:, bslot + jh * 4 + j4, :],
                            ident[:],
                        )
                    nc.scalar.copy(awt[:, jh * 512:(jh + 1) * 512], psdt[:])

                for cp in range(4):     # slot pair (2*cp, 2*cp+1)
                    ps3 = ps3p.tile([128, 512], f32, tag="ps3")
                    for c2 in range(2):
                        c = 2 * cp + c2
                        nc.tensor.matmul(
                            ps3[:, c2 * 256:(c2 + 1) * 256],
                            awt[:, c * 128:(c + 1) * 128],
                            xp[:, c, :],
                            start=True, stop=True,
                        )
                    osb = osbp.tile([128, 512], f32, tag="osb")
                    nc.scalar.activation(osb[:], ps3[:], ACTF.Exp)
                    osb3 = osb[:].rearrange("p (jj e) -> p jj e", jj=2)
                    nc.sync.dma_start(
                        o_d[base + 2 * cp:base + 2 * cp + 2].rearrange(
                            "jj f e -> f jj e"),
                        osb3[0:64],
                    )
                    nc.sync.dma_start(
                        o_d[base + 8 + 2 * cp:base + 8 + 2 * cp + 2].rearrange(
                            "jj f e -> f jj e"),
                        osb3[64:128],
                    )

            # ---------------- software pipeline over groups ----------------
            pend = None
            for g in range(NG):
                xs_g = xsgp.tile([128, G, F], f32, tag="xsg")
                aw2 = aw2_bufs[g % 2]
                xps = []
                for bk in range(BPG):
                    bank = g * BPG + bk
                    xps.append(phase_a(bank, xs_g))
                entmax(xs_g, aw2)
                if pend is not None:
                    pbanks, paw, pxps = pend
                    for bk in range(BPG):
                        phase_c(pbanks[bk], paw, pxps[bk])
                pend = ([g * BPG + bk for bk in range(BPG)], aw2[:], xps)
            pbanks, paw, pxps = pend
            for bk in range(BPG):
                phase_c(pbanks[bk], paw, pxps[bk])

    if not nc.is_finalized():
        nc.finalize()
    return nc


_NC_CACHE = {}


def _get_program(B_loc):
    key = B_loc
    if key not in _NC_CACHE:
        _NC_CACHE[key] = build_program(B_loc)
    return _NC_CACHE[key]


def kernel(**inputs):
    from concourse.bass_utils import run_bass_kernel_spmd

    x = np.ascontiguousarray(np.asarray(inputs["x"], dtype=np.float32))
    w = np.ascontiguousarray(np.asarray(inputs["bilinear_w"], dtype=np.float32))
    q = np.ascontiguousarray(np.asarray(inputs["query"], dtype=np.float32))
    v = np.ascontiguousarray(np.asarray(inputs["value"], dtype=np.float32))
    B = x.shape[0]
    B_loc = B // NCORES

    nc = _get_program(B_loc)

    in_maps = []
    for core in range(NCORES):
        sh = x[core * B_loc:(core + 1) * B_loc]
        in_maps.append(
            {"x": np.ascontiguousarray(sh), "bilinear_w": w, "query": q, "value": v}
        )

    import os
    trace = bool(int(os.environ.get("KERNEL_TRACE", "0")))
    res = run_bass_kernel_spmd(
        nc, in_maps, core_ids=list(range(NCORES)), trace=trace,
        trace_cores=[0] if trace else None,
    )
    if trace:
        kernel.last_exec_time_ns = res.exec_time_ns
        kernel.last_trace = res.instructions_and_trace
    out = np.concatenate([r["out"] for r in res.results], axis=0)
    return out


if __name__ == "__main__":
    nc = build_program(B_LOC)
    print("build ok:", len(nc.inst_map), "instructions")


# revision 12
# speedup vs baseline: 2.6567x; 1.0121x over previous
"""Trainium2 Bass kernel for nn_CrossFeature (sparse_attention).

Math (per batch b):
    att[b,n,f]  = (x[b] @ W.T @ q.T).T * E**-0.5
    Xs          = 0.5 * att                               # entmax15 pre-scale
    gate        = entmax15(att) over f  (Newton on the entmax root)
    out[b,n,e]  = exp( sum_f gate*value * x[b,f,e] )

v3 design: all-fp32 storage, float32r (TF32-ish) matmuls (no bf16
casts); x transposed on the PE instead of DMA-transpose; stage-1/2 as
512-col moving matmuls; entmax Newton on whole-group [128, 32*64] DVE
passes with segmented tensor_reduce instead of per-pair bn_stats.

fp32r matmuls require dst partition base 0 (no tile_position), so:
  * stage-1/2 routes the two partition halves of the Xs PSUM bank via
    zero-padded stationaries [qt|0] / [0|qt] and full-height matmuls;
  * each bank pairs batch T_c=base+c (partitions 0:64) with
    B_c=base+8+c (64:128); the gate lives in a zero-interleaved tile
    aw2[0:64, slot, 0:64]=gate(T) / [64:128, slot, 64:128]=gate(B);
    its PE transpose is block-diagonal [gT(T),0;0,gT(B)] so one
    full-height fp32r matmul computes stage-3 for both batches.
  * 1/s2 is folded into the gate so stage-3 exp needs no per-batch
    scale and runs on [128,512].

Sharding: pure data-parallel, batch 2048 -> 8 cores x 256.
"""

import numpy as np

B_FULL, F, E, N = 2048, 64, 256, 64
NCORES = 8
B_LOC = B_FULL // NCORES

SCALE = 0.5 * (E ** -0.5)   # folds entmax's (alpha-1) into qtilde
CBAR = 0.097                # linearization point for sqrt((1-v64)/64)
NEWTON_ITERS = 2


def build_program(B_loc=B_LOC, newton_iters=NEWTON_ITERS):
    import concourse.tile as tile
    from concourse import bacc, mybir, masks

    f32 = mybir.dt.float32
    f32r = mybir.dt.float32r
    bf16g = None  # set below
    Alu = mybir.AluOpType
    ACTF = mybir.ActivationFunctionType
    AX = mybir.AxisListType

    NBANKS = B_loc // 16          # 16 batches per bank (8 T + 8 B)
    NG = 4                        # groups (entmax granularity)
    BPG = NBANKS // NG            # banks per group
    G = BPG * 8                   # batch-slots per group tile (32)
    assert NBANKS % NG == 0

    K0 = 0.5 * CBAR + 1.0 / (128.0 * CBAR)
    KW = 1.0 / (128.0 * CBAR)

    nc = bacc.Bacc("TRN2", debug=False, num_devices=NCORES)
    x_d = nc.dram_tensor("x", [B_loc, F, E], f32r, kind="ExternalInput").ap()
    w_d = nc.dram_tensor("bilinear_w", [E, E], f32, kind="ExternalInput").ap()
    q_d = nc.dram_tensor("query", [N, E], f32, kind="ExternalInput").ap()
    v_d = nc.dram_tensor("value", [N, F], f32, kind="ExternalInput").ap()
    o_d = nc.dram_tensor("out", [B_loc, N, E], f32, kind="ExternalOutput").ap()

    # batch (bb, r, jj) = bb*16 + r*8 + jj; partition pair = (T_jj | B_jj)


    with tile.TileContext(nc) as tc:
        with (
            tc.tile_pool(name="const", bufs=1) as constp,
            tc.tile_pool(name="xp", bufs=4) as xpp,       # [128,8,256]
            tc.tile_pool(name="xt", bufs=4) as xtp,       # [128,1024] x(2/bank)
            tc.tile_pool(name="xsg", bufs=2) as xsgp,     # [128,G,64]
            tc.tile_pool(name="scr", bufs=2) as scrp,     # r chain
            tc.tile_pool(name="aw2", bufs=2) as aw2p,     # zero-interleaved gate
            tc.tile_pool(name="sm", bufs=2) as smp,       # [128,G] smalls
            tc.tile_pool(name="awt", bufs=3) as awtp,     # [128,1024]
            tc.tile_pool(name="osb", bufs=6) as osbp,     # [128,512]
            tc.tile_pool(name="pst", bufs=2, space="PSUM") as pstp,
            tc.tile_pool(name="ps12", bufs=2, space="PSUM") as ps12p,
            tc.tile_pool(name="psdt", bufs=2, space="PSUM") as psdtp,
            tc.tile_pool(name="ps3", bufs=2, space="PSUM") as ps3p,
        ):
            # ---------------- constants ----------------
            ident = constp.tile([128, 128], f32)
            masks.make_identity(nc, ident[:])
            identr = constp.tile([128, 128], f32r, tag="identr")
            nc.scalar.copy(identr[:], ident[:])

            bf16 = mybir.dt.bfloat16
            v2 = constp.tile([128, F], f32)
            nc.sync.dma_start(v2[0:64, :], v_d[:, :])
            nc.sync.dma_start(v2[64:128, :], v_d[:, :])
            v2bf = constp.tile([128, F], bf16, tag="v2bf")
            nc.scalar.copy(v2bf[:], v2[:])

            wt = {}
            for di in range(2):
                for ej in range(2):
                    t = constp.tile([128, 128], f32, tag=f"wt{di}{ej}")
                    nc.sync.dma_start(
                        t[:], w_d[di * 128:(di + 1) * 128, ej * 128:(ej + 1) * 128]
                    )
                    wt[di, ej] = t

            qtin = []
            for di in range(2):
                t = constp.tile([128, N], f32, tag=f"qtin{di}")
                nc.sync.dma_start(
                    t[:], q_d[:, di * 128:(di + 1) * 128].transpose([1, 0])
                )
                qtin.append(t)

            # qtilde^T[e, n] = sum_d W[d, e] q[n, d], scaled.
            # qtz[ec][h]: [128,128] stationary, h=0 -> [qt|0], h=1 -> [0|qt]
            qtz = []
            for ej in range(2):
                ps = ps12p.tile([128, 512], f32, tag="ps12")
                for di in range(2):
                    nc.tensor.matmul(
                        ps[:, 0:N], wt[di, ej][:], qtin[di][:],
                        start=(di == 0), stop=(di == 1),
                    )
                pair = []
                for h in range(2):
                    t = constp.tile([128, 128], f32r, tag=f"qtz{ej}{h}")
                    nc.scalar.mul(t[:, h * 64:h * 64 + 64], ps[:, 0:N], SCALE)
                    nc.scalar.mul(t[:, (1 - h) * 64:(1 - h) * 64 + 64],
                                  ps[:, 0:N], 0.0)
                    pair.append(t)
                qtz.append(pair)

            # aw2 buffers: zero-interleaved gate; off-quadrants stay 0 forever
            aw2_bufs = []
            for i in range(2):
                t = aw2p.tile([128, G, 128], f32, tag="aw2")
                nc.vector.memset(t[0:64, :, 64:128], 0.0)
                nc.vector.memset(t[64:128, :, 0:64], 0.0)
                aw2_bufs.append(t)

            # ---------------- per-bank phase A ----------------
            def phase_a(bank, xs_g):
                bslot = (bank % BPG) * 8
                base = bank * 16
                xp = xpp.tile([128, 8, 256], f32r, tag="xp")
                nc.sync.dma_start(
                    xp[0:64, :, :],
                    x_d[base:base + 8].rearrange("jj f e -> f jj e"),
                )
                nc.sync.dma_start(
                    xp[64:128, :, :],
                    x_d[base + 8:base + 16].rearrange("jj f e -> f jj e"),
                )

                xt = []
                for ec in range(2):
                    t = xtp.tile([128, 1024], f32r, tag="xt")
                    t4 = t[:].rearrange("p (tb jh c) -> p tb jh c", tb=2, jh=2)
                    for jh in range(2):
                        pst = pstp.tile([128, 512], f32r, tag="pst")
                        pst4 = pst[:].rearrange(
                            "p (tb c f) -> p tb c f", tb=2, c=4
                        )
                        for j4 in range(4):
                            jj = jh * 4 + j4
                            nc.tensor.transpose(
                                pst4[:, :, j4, :],
                                xp[:, jj, ec * 128:(ec + 1) * 128],
                                identr[:],
                            )
                        nc.scalar.copy(t4[:, :, jh, :], pst[:])
                    xt.append(t)

                ps = ps12p.tile([128, 512], f32, tag="ps12")
                k = 0
                for h in range(2):
                    for ec in range(2):
                        nc.tensor.matmul(
                            ps[:, :],
                            qtz[ec][h][:],
                            xt[ec][:, h * 512:(h + 1) * 512],
                            start=(k == 0), stop=(k == 3),
                        )
                        k += 1
                nc.scalar.copy(
                    xs_g[:, bslot:bslot + 8, :],
                    ps[:].rearrange("p (s f) -> p s f", f=F),
                )
                return xp

            # ---------------- entmax (per group) ----------------
            def entmax(xs_g, aw2):
                xs3 = xs_g[:]
                bf16 = mybir.dt.bfloat16
                r_t = scrp.tile([128, G, F], bf16, tag="r")
                r = r_t[:]
                sr = smp.tile([128, G], f32, tag="sr")
                srq = smp.tile([128, G], f32, tag="srq")
                tau = smp.tile([128, G], f32, tag="tau")
                s1 = smp.tile([128, G], f32, tag="s1")
                u = smp.tile([128, G], f32, tag="u")
                w = smp.tile([128, G], f32, tag="w")
                s2 = smp.tile([128, G], f32, tag="s2")
                rec = smp.tile([128, G], f32, tag="rec")
                dlt = smp.tile([128, G], f32, tag="dlt")
                tau_bf = smp.tile([128, G], bf16, tag="taubf")
                rec_bf = smp.tile([128, G], bf16, tag="recbf")
                taub = tau_bf[:].unsqueeze(2).broadcast_to([128, G, F])

                def seg_sum(dst, src3):
                    nc.vector.tensor_reduce(
                        dst[:], src3, axis=AX.X, op=Alu.add,
                    )

                # init: tau0 = sr/64 + KW*(srq - sr^2/64) - K0
                nc.vector.tensor_mul(r, xs3, xs3)
                seg_sum(srq, r)
                seg_sum(sr, xs3)
                nc.vector.tensor_mul(u[:], sr[:], sr[:])
                nc.vector.scalar_tensor_tensor(
                    out=w[:], in0=u[:], scalar=-1.0 / 64.0, in1=srq[:],
                    op0=Alu.mult, op1=Alu.add,
                )  # w = V = srq - sr^2/64
                nc.vector.tensor_scalar(
                    out=w[:], in0=w[:], scalar1=KW, scalar2=-K0,
                    op0=Alu.mult, op1=Alu.add,
                )  # w = KW*V - K0
                nc.vector.scalar_tensor_tensor(
                    out=tau[:], in0=sr[:], scalar=1.0 / 64.0, in1=w[:],
                    op0=Alu.mult, op1=Alu.add,
                )

                for _ in range(newton_iters):
                    nc.vector.tensor_copy(tau_bf[:], tau[:])
                    nc.vector.tensor_max(r, xs3, taub)
                    seg_sum(sr, r)
                    nc.vector.tensor_mul(r, r, r)
                    seg_sum(srq, r)
                    # s1 = sr - 64*tau ; s2 = srq - 2*tau*sr + 64*tau^2
                    nc.vector.scalar_tensor_tensor(
                        out=s1[:], in0=tau[:], scalar=-64.0, in1=sr[:],
                        op0=Alu.mult, op1=Alu.add,
                    )
                    nc.vector.tensor_mul(u[:], tau[:], sr[:])
                    nc.vector.tensor_mul(w[:], tau[:], tau[:])
                    nc.vector.scalar_tensor_tensor(
                        out=s2[:], in0=w[:], scalar=64.0, in1=srq[:],
                        op0=Alu.mult, op1=Alu.add,
                    )
                    nc.vector.scalar_tensor_tensor(
                        out=s2[:], in0=u[:], scalar=-2.0, in1=s2[:],
                        op0=Alu.mult, op1=Alu.add,
                    )
                    # tau += (s2 - 1) / (2*s1)
                    nc.vector.reciprocal(rec[:], s1[:])
                    nc.vector.tensor_scalar(
                        out=s2[:], in0=s2[:], scalar1=-1.0, scalar2=None,
                        op0=Alu.add,
                    )
                    nc.vector.tensor_mul(dlt[:], s2[:], rec[:])
                    nc.vector.scalar_tensor_tensor(
                        out=tau[:], in0=dlt[:], scalar=0.5, in1=tau[:],
                        op0=Alu.mult, op1=Alu.add,
                    )

                # final: d = relu(Xs - tau); aw = d^2 * v / s2, written into
                # the data quadrants of the zero-interleaved aw2
                nc.vector.tensor_copy(tau_bf[:], tau[:])
                nc.vector.tensor_max(r, xs3, taub)
                nc.vector.tensor_sub(r, r, taub)        # d
                nc.vector.tensor_mul(r, r, r)           # d^2
                seg_sum(s2, r)
                nc.vector.reciprocal(rec[:], s2[:])
                nc.vector.tensor_copy(rec_bf[:], rec[:])
                v2b = v2bf[:].unsqueeze(1).broadcast_to([128, G, F])
                nc.vector.tensor_mul(r, r, v2b)         # d^2 * v
                recb = rec_bf[:].unsqueeze(2).broadcast_to([128, G, F])
                aw3 = aw2[:]
                nc.vector.tensor_mul(
                    aw3[0:64, :, 0:64], r[0:64, :, :], recb[0:64, :, :]
                )
                nc.vector.tensor_mul(
                    aw3[64:128, :, 64:128], r[64:128, :, :], recb[64:128, :, :]
                )

            # ---------------- per-bank phase C ----------------
            def phase_c(bank, aw2, xp):
                base = bank * 16
                bslot = (bank % BPG) * 8
                awt = awtp.tile([128, 1024], f32r, tag="awt")
                for jh in range(2):
                    psdt = psdtp.tile([128, 512], f32, tag="psdt")
                    for j4 in range(4):
                        nc.tensor.transpose(
                            psdt[:, j4 * 128:(j4 + 1) * 128],
                            aw2[:, bslot + jh * 4 + j4, :],
                            ident[:],
                        )
                    nc.scalar.copy(awt[:, jh * 512:(jh + 1) * 512], psdt[:])

                for cp in range(4):     # slot pair (2*cp, 2*cp+1)
                    ps3 = ps3p.tile([128, 512], f32, tag="ps3")
                    for c2 in range(2):
                        c = 2 * cp + c2
                        nc.tensor.matmul(
                            ps3[:, c2 * 256:(c2 + 1) * 256],
                            awt[:, c * 128:(c + 1) * 128],
                            xp[:, c, :],
                            start=True, stop=True,
                        )
                    osb = osbp.tile([128, 512], f32, tag="osb")
                    nc.scalar.activation(osb[:], ps3[:], ACTF.Exp)
                    osb3 = osb[:].rearrange("p (jj e) -> p jj e", jj=2)
                    nc.sync.dma_start(
                        o_d[base + 2 * cp:base + 2 * cp + 2].rearrange(
                            "jj f e -> f jj e"),
                        osb3[0:64],
                    )
                    nc.sync.dma_start(
                        o_d[base + 8 + 2 * cp:base + 8 + 2 * cp + 2].rearrange(
                            "jj f e -> f jj e"),
                        osb3[64:128],
                    )

            # ---------------- software pipeline over groups ----------------
            pend = None
            for g in range(NG):
                xs_g = xsgp.tile([128, G, F], bf16, tag="xsg")
                aw2 = aw2_bufs[g % 2]
                xps = []
                for bk in range(BPG):
                    bank = g * BPG + bk
                    xps.append(phase_a(bank, xs_g))
                entmax(xs_g, aw2)
                if pend is not None:
                    pbanks, paw, pxps = pend
                    for bk in range(BPG):
                        phase_c(pbanks[bk], paw, pxps[bk])
                pend = ([g * BPG + bk for bk in range(BPG)], aw2[:], xps)
            pbanks, paw, pxps = pend
            for bk in range(BPG):
                phase_c(pbanks[bk], paw, pxps[bk])

    if not nc.is_finalized():
        nc.finalize()
    return nc


_NC_CACHE = {}


def _get_program(B_loc):
    key = B_loc
    if key not in _NC_CACHE:
        _NC_CACHE[key] = build_program(B_loc)
    return _NC_CACHE[key]


def kernel(**inputs):
    from concourse.bass_utils import run_bass_kernel_spmd

    x = np.ascontiguousarray(np.asarray(inputs["x"], dtype=np.float32))
    w = np.ascontiguousarray(np.asarray(inputs["bilinear_w"], dtype=np.float32))
    q = np.ascontiguousarray(np.asarray(inputs["query"], dtype=np.float32))
    v = np.ascontiguousarray(np.asarray(inputs["value"], dtype=np.float32))
    B = x.shape[0]
    B_loc = B // NCORES

    nc = _get_program(B_loc)

    in_maps = []
    for core in range(NCORES):
        sh = x[core * B_loc:(core + 1) * B_loc]
        in_maps.append(
            {"x": np.ascontiguousarray(sh), "bilinear_w": w, "query": q, "value": v}
        )

    import os
    trace = bool(int(os.environ.get("KERNEL_TRACE", "0")))
    res = run_bass_kernel_spmd(
        nc, in_maps, core_ids=list(range(NCORES)), trace=trace,
        trace_cores=[0] if trace else None,
    )
    if trace:
        kernel.last_exec_time_ns = res.exec_time_ns
        kernel.last_trace = res.instructions_and_trace
    out = np.concatenate([r["out"] for r in res.results], axis=0)
    return out


if __name__ == "__main__":
    nc = build_program(B_LOC)
    print("build ok:", len(nc.inst_map), "instructions")


# revision 13
# speedup vs baseline: 3.7739x; 1.4205x over previous
"""Trainium2 Bass kernel for nn_CrossFeature (sparse_attention).

Math (per batch b):
    att[b,n,f]  = (x[b] @ W.T @ q.T).T * E**-0.5
    Xs          = 0.5 * att                               # entmax15 pre-scale
    gate        = entmax15(att) over f  (Newton on the entmax root)
    out[b,n,e]  = exp( sum_f gate*value * x[b,f,e] )

v3 design: all-fp32 storage, float32r (TF32-ish) matmuls (no bf16
casts); x transposed on the PE instead of DMA-transpose; stage-1/2 as
512-col moving matmuls; entmax Newton on whole-group [128, 32*64] DVE
passes with segmented tensor_reduce instead of per-pair bn_stats.

fp32r matmuls require dst partition base 0 (no tile_position), so:
  * stage-1/2 routes the two partition halves of the Xs PSUM bank via
    zero-padded stationaries [qt|0] / [0|qt] and full-height matmuls;
  * each bank pairs batch T_c=base+c (partitions 0:64) with
    B_c=base+8+c (64:128); the gate lives in a zero-interleaved tile
    aw2[0:64, slot, 0:64]=gate(T) / [64:128, slot, 64:128]=gate(B);
    its PE transpose is block-diagonal [gT(T),0;0,gT(B)] so one
    full-height fp32r matmul computes stage-3 for both batches.
  * 1/s2 is folded into the gate so stage-3 exp needs no per-batch
    scale and runs on [128,512].

Sharding: pure data-parallel, batch 2048 -> 8 cores x 256.
"""

import numpy as np

B_FULL, F, E, N = 2048, 64, 256, 64
NCORES = 8
B_LOC = B_FULL // NCORES

SCALE = 0.5 * (E ** -0.5)   # folds entmax's (alpha-1) into qtilde
CBAR = 0.097                # linearization point for sqrt((1-v64)/64)
NEWTON_ITERS = 2


def build_program(B_loc=B_LOC, newton_iters=NEWTON_ITERS):
    import concourse.tile as tile
    from concourse import bacc, mybir, masks

    f32 = mybir.dt.float32
    f32r = mybir.dt.float32r
    bf16g = None  # set below
    Alu = mybir.AluOpType
    ACTF = mybir.ActivationFunctionType
    AX = mybir.AxisListType

    NBANKS = B_loc // 16          # 16 batches per bank (8 T + 8 B)
    NG = 4                        # groups (entmax granularity)
    BPG = NBANKS // NG            # banks per group
    G = BPG * 8                   # batch-slots per group tile (32)
    assert NBANKS % NG == 0

    K0 = 0.5 * CBAR + 1.0 / (128.0 * CBAR)
    KW = 1.0 / (128.0 * CBAR)

    nc = bacc.Bacc("TRN2", debug=False, num_devices=NCORES)
    x_d = nc.dram_tensor("x", [B_loc, F, E], f32r, kind="ExternalInput").ap()
    w_d = nc.dram_tensor("bilinear_w", [E, E], f32, kind="ExternalInput").ap()
    q_d = nc.dram_tensor("query", [N, E], f32, kind="ExternalInput").ap()
    v_d = nc.dram_tensor("value", [N, F], f32, kind="ExternalInput").ap()
    o_d = nc.dram_tensor("out", [B_loc, N, E], f32, kind="ExternalOutput").ap()

    # batch (bb, r, jj) = bb*16 + r*8 + jj; partition pair = (T_jj | B_jj)


    with tile.TileContext(nc) as tc:
        with (
            tc.tile_pool(name="const", bufs=1) as constp,
            tc.tile_pool(name="xp", bufs=8) as xpp,       # [128,8,256]
            tc.tile_pool(name="xt", bufs=6) as xtp,       # [128,1024] x(2/bank)
            tc.tile_pool(name="xsg", bufs=2) as xsgp,     # [128,G,64]
            tc.tile_pool(name="scr", bufs=2) as scrp,     # r chain
            tc.tile_pool(name="aw2", bufs=2) as aw2p,     # zero-interleaved gate
            tc.tile_pool(name="sm", bufs=2) as smp,       # [128,G] smalls
            tc.tile_pool(name="awt", bufs=4) as awtp,     # [128,1024]
            tc.tile_pool(name="osb", bufs=6) as osbp,     # [128,512]
            tc.tile_pool(name="pst", bufs=2, space="PSUM") as pstp,
            tc.tile_pool(name="ps12", bufs=2, space="PSUM") as ps12p,
            tc.tile_pool(name="psdt", bufs=2, space="PSUM") as psdtp,
            tc.tile_pool(name="ps3", bufs=2, space="PSUM") as ps3p,
        ):
            # ---------------- constants ----------------
            ident = constp.tile([128, 128], f32)
            masks.make_identity(nc, ident[:])
            identr = constp.tile([128, 128], f32r, tag="identr")
            nc.scalar.copy(identr[:], ident[:])

            bf16 = mybir.dt.bfloat16
            v2 = constp.tile([128, F], f32)
            nc.sync.dma_start(v2[0:64, :], v_d[:, :])
            nc.sync.dma_start(v2[64:128, :], v_d[:, :])
            v2bf = constp.tile([128, F], bf16, tag="v2bf")
            nc.scalar.copy(v2bf[:], v2[:])

            wt = {}
            for di in range(2):
                for ej in range(2):
                    t = constp.tile([128, 128], f32, tag=f"wt{di}{ej}")
                    nc.sync.dma_start(
                        t[:], w_d[di * 128:(di + 1) * 128, ej * 128:(ej + 1) * 128]
                    )
                    wt[di, ej] = t

            qtin = []
            for di in range(2):
                t = constp.tile([128, N], f32, tag=f"qtin{di}")
                nc.sync.dma_start(
                    t[:], q_d[:, di * 128:(di + 1) * 128].transpose([1, 0])
                )
                qtin.append(t)

            # qtilde^T[e, n] = sum_d W[d, e] q[n, d], scaled.
            # qtz[ec][h]: [128,128] stationary, h=0 -> [qt|0], h=1 -> [0|qt]
            qtz = []
            for ej in range(2):
                ps = ps12p.tile([128, 512], f32, tag="ps12")
                for di in range(2):
                    nc.tensor.matmul(
                        ps[:, 0:N], wt[di, ej][:], qtin[di][:],
                        start=(di == 0), stop=(di == 1),
                    )
                pair = []
                for h in range(2):
                    t = constp.tile([128, 128], f32r, tag=f"qtz{ej}{h}")
                    nc.scalar.mul(t[:, h * 64:h * 64 + 64], ps[:, 0:N], SCALE)
                    nc.scalar.mul(t[:, (1 - h) * 64:(1 - h) * 64 + 64],
                                  ps[:, 0:N], 0.0)
                    pair.append(t)
                qtz.append(pair)

            # aw2 buffers: zero-interleaved gate; off-quadrants stay 0 forever
            aw2_bufs = []
            for i in range(2):
                t = aw2p.tile([128, G, 128], f32, tag="aw2")
                nc.vector.memset(t[0:64, :, 64:128], 0.0)
                nc.vector.memset(t[64:128, :, 0:64], 0.0)
                aw2_bufs.append(t)

            # ---------------- per-bank phase A ----------------
            def phase_a(bank, xs_g):
                bslot = (bank % BPG) * 8
                base = bank * 16
                xp = xpp.tile([128, 8, 256], f32r, tag="xp")
                nc.sync.dma_start(
                    xp[0:64, :, :],
                    x_d[base:base + 8].rearrange("jj f e -> f jj e"),
                )
                nc.sync.dma_start(
                    xp[64:128, :, :],
                    x_d[base + 8:base + 16].rearrange("jj f e -> f jj e"),
                )

                xt = []
                for ec in range(2):
                    t = xtp.tile([128, 1024], f32r, tag="xt")
                    t4 = t[:].rearrange("p (tb jh c) -> p tb jh c", tb=2, jh=2)
                    for jh in range(2):
                        pst = pstp.tile([128, 512], f32r, tag="pst")
                        pst4 = pst[:].rearrange(
                            "p (tb c f) -> p tb c f", tb=2, c=4
                        )
                        for j4 in range(4):
                            jj = jh * 4 + j4
                            nc.tensor.transpose(
                                pst4[:, :, j4, :],
                                xp[:, jj, ec * 128:(ec + 1) * 128],
                                identr[:],
                            )
                        nc.scalar.copy(t4[:, :, jh, :], pst[:])
                    xt.append(t)

                ps = ps12p.tile([128, 512], f32, tag="ps12")
                k = 0
                for h in range(2):
                    for ec in range(2):
                        nc.tensor.matmul(
                            ps[:, :],
                            qtz[ec][h][:],
                            xt[ec][:, h * 512:(h + 1) * 512],
                            start=(k == 0), stop=(k == 3),
                        )
                        k += 1
                nc.scalar.copy(
                    xs_g[:, bslot:bslot + 8, :],
                    ps[:].rearrange("p (s f) -> p s f", f=F),
                )
                return xp

            # ---------------- entmax (per group) ----------------
            def entmax(xs_g, aw2):
                xs3 = xs_g[:]
                bf16 = mybir.dt.bfloat16
                r_t = scrp.tile([128, G, F], bf16, tag="r")
                r = r_t[:]
                sr = smp.tile([128, G], f32, tag="sr")
                srq = smp.tile([128, G], f32, tag="srq")
                tau = smp.tile([128, G], f32, tag="tau")
                s1 = smp.tile([128, G], f32, tag="s1")
                u = smp.tile([128, G], f32, tag="u")
                w = smp.tile([128, G], f32, tag="w")
                s2 = smp.tile([128, G], f32, tag="s2")
                rec = smp.tile([128, G], f32, tag="rec")
                dlt = smp.tile([128, G], f32, tag="dlt")
                tau_bf = smp.tile([128, G], bf16, tag="taubf")
                rec_bf = smp.tile([128, G], bf16, tag="recbf")
                taub = tau_bf[:].unsqueeze(2).broadcast_to([128, G, F])

                def seg_sum(dst, src3):
                    nc.vector.tensor_reduce(
                        dst[:], src3, axis=AX.X, op=Alu.add,
                    )

                # init: tau0 = sr/64 + KW*(srq - sr^2/64) - K0
                nc.vector.tensor_mul(r, xs3, xs3)
                seg_sum(srq, r)
                seg_sum(sr, xs3)
                nc.vector.tensor_mul(u[:], sr[:], sr[:])
                nc.vector.scalar_tensor_tensor(
                    out=w[:], in0=u[:], scalar=-1.0 / 64.0, in1=srq[:],
                    op0=Alu.mult, op1=Alu.add,
                )  # w = V = srq - sr^2/64
                nc.vector.tensor_scalar(
                    out=w[:], in0=w[:], scalar1=KW, scalar2=-K0,
                    op0=Alu.mult, op1=Alu.add,
                )  # w = KW*V - K0
                nc.vector.scalar_tensor_tensor(
                    out=tau[:], in0=sr[:], scalar=1.0 / 64.0, in1=w[:],
                    op0=Alu.mult, op1=Alu.add,
                )

                for _ in range(newton_iters):
                    nc.vector.tensor_copy(tau_bf[:], tau[:])
                    nc.vector.tensor_max(r, xs3, taub)
                    seg_sum(sr, r)
                    nc.vector.tensor_mul(r, r, r)
                    seg_sum(srq, r)
                    # s1 = sr - 64*tau ; s2 = srq - 2*tau*sr + 64*tau^2
                    nc.vector.scalar_tensor_tensor(
                        out=s1[:], in0=tau[:], scalar=-64.0, in1=sr[:],
                        op0=Alu.mult, op1=Alu.add,
                    )
                    nc.vector.tensor_mul(u[:], tau[:], sr[:])
                    nc.vector.tensor_mul(w[:], tau[:], tau[:])
                    nc.vector.scalar_tensor_tensor(
                        out=s2[:], in0=w[:], scalar=64.0, in1=srq[:],
                        op0=Alu.mult, op1=Alu.add,
                    )
                    nc.vector.scalar_tensor_tensor(
                        out=s2[:], in0=u[:], scalar=-2.0, in1=s2[:],
                        op0=Alu.mult, op1=Alu.add,
                    )
                    # tau += (s2 - 1) / (2*s1)
                    nc.vector.reciprocal(rec[:], s1[:])
                    nc.vector.tensor_scalar(
                        out=s2[:], in0=s2[:], scalar1=-1.0, scalar2=None,
                        op0=Alu.add,
                    )
                    nc.vector.tensor_mul(dlt[:], s2[:], rec[:])
                    nc.vector.scalar_tensor_tensor(
                        out=tau[:], in0=dlt[:], scalar=0.5, in1=tau[:],
                        op0=Alu.mult, op1=Alu.add,
                    )

                # final: d = relu(Xs - tau); aw = d^2 * v / s2, written into
                # the data quadrants of the zero-interleaved aw2
                nc.vector.tensor_copy(tau_bf[:], tau[:])
                nc.vector.tensor_max(r, xs3, taub)
                nc.vector.tensor_sub(r, r, taub)        # d
                nc.vector.tensor_mul(r, r, r)           # d^2
                seg_sum(s2, r)
                nc.vector.reciprocal(rec[:], s2[:])
                nc.vector.tensor_copy(rec_bf[:], rec[:])
                v2b = v2bf[:].unsqueeze(1).broadcast_to([128, G, F])
                nc.vector.tensor_mul(r, r, v2b)         # d^2 * v
                recb = rec_bf[:].unsqueeze(2).broadcast_to([128, G, F])
                aw3 = aw2[:]
                nc.vector.tensor_mul(
                    aw3[0:64, :, 0:64], r[0:64, :, :], recb[0:64, :, :]
                )
                nc.vector.tensor_mul(
                    aw3[64:128, :, 64:128], r[64:128, :, :], recb[64:128, :, :]
                )

            # ---------------- per-bank phase C ----------------
            def phase_c(bank, aw2, xp):
                base = bank * 16
                bslot = (bank % BPG) * 8
                awt = awtp.tile([128, 1024], f32r, tag="awt")
                for jh in range(2):
                    psdt = psdtp.tile([128, 512], f32, tag="psdt")
                    for j4 in range(4):
                        nc.tensor.transpose(
                            psdt[:, j4 * 128:(j4 + 1) * 128],
                            aw2[:, bslot + jh * 4 + j4, :],
                            ident[:],
                        )
                    nc.scalar.copy(awt[:, jh * 512:(jh + 1) * 512], psdt[:])

                for cp in range(4):     # slot pair (2*cp, 2*cp+1)
                    ps3 = ps3p.tile([128, 512], f32, tag="ps3")
                    for c2 in range(2):
                        c = 2 * cp + c2
                        nc.tensor.matmul(
                            ps3[:, c2 * 256:(c2 + 1) * 256],
                            awt[:, c * 128:(c + 1) * 128],
                            xp[:, c, :],
                            start=True, stop=True,
                        )
                    osb = osbp.tile([128, 512], f32, tag="osb")
                    nc.scalar.activation(osb[:], ps3[:], ACTF.Exp)
                    osb3 = osb[:].rearrange("p (jj e) -> p jj e", jj=2)
                    nc.sync.dma_start(
                        o_d[base + 2 * cp:base + 2 * cp + 2].rearrange(
                            "jj f e -> f jj e"),
                        osb3[0:64],
                    )
                    nc.sync.dma_start(
                        o_d[base + 8 + 2 * cp:base + 8 + 2 * cp + 2].rearrange(
                            "jj f e -> f jj e"),
                        osb3[64:128],
                    )

            # ---------------- software pipeline over groups ----------------
            pend = None
            for g in range(NG):
                xs_g = xsgp.tile([128, G, F], bf16, tag="xsg")
                aw2 = aw2_bufs[g % 2]
                xps = []
                for bk in range(BPG):
                    bank = g * BPG + bk
                    xps.append(phase_a(bank, xs_g))
                entmax(xs_g, aw2)
                if pend is not None:
                    pbanks, paw, pxps = pend
                    for bk in range(BPG):
                        phase_c(pbanks[bk], paw, pxps[bk])
                pend = ([g * BPG + bk for bk in range(BPG)], aw2[:], xps)
            pbanks, paw, pxps = pend
            for bk in range(BPG):
                phase_c(pbanks[bk], paw, pxps[bk])

    if not nc.is_finalized():
        nc.finalize()
    return nc


_NC_CACHE = {}


def _get_program(B_loc):
    key = B_loc
    if key not in _NC_CACHE:
        _NC_CACHE[key] = build_program(B_loc)
    return _NC_CACHE[key]


def kernel(**inputs):
    from concourse.bass_utils import run_bass_kernel_spmd

    x = np.ascontiguousarray(np.asarray(inputs["x"], dtype=np.float32))
    w = np.ascontiguousarray(np.asarray(inputs["bilinear_w"], dtype=np.float32))
    q = np.ascontiguousarray(np.asarray(inputs["query"], dtype=np.float32))
    v = np.ascontiguousarray(np.asarray(inputs["value"], dtype=np.float32))
    B = x.shape[0]
    B_loc = B // NCORES

    nc = _get_program(B_loc)

    in_maps = []
    for core in range(NCORES):
        sh = x[core * B_loc:(core + 1) * B_loc]
        in_maps.append(
            {"x": np.ascontiguousarray(sh), "bilinear_w": w, "query": q, "value": v}
        )

    import os
    trace = bool(int(os.environ.get("KERNEL_TRACE", "0")))
    res = run_bass_kernel_spmd(
        nc, in_maps, core_ids=list(range(NCORES)), trace=trace,
        trace_cores=[0] if trace else None,
    )
    if trace:
        kernel.last_exec_time_ns = res.exec_time_ns
        kernel.last_trace = res.instructions_and_trace
    out = np.concatenate([r["out"] for r in res.results], axis=0)
    return out


if __name__ == "__main__":
    nc = build_program(B_LOC)
    print("build ok:", len(nc.inst_map), "instructions")
